# revision 5
# baseline (speedup 1.0000x reference)
"""ContinuousWaveletTransform (Morlet bank, 32 scales) on 8 TRN2 cores.

Key structure exploited: the reference wavelet is w[k] = exp(-0.5 k^2) *
exp(i 2pi k / 6) (bandwidth=1), so the envelope underflows to exactly 0.0f
after k=14, and taps k>=7 are < 2.3e-11.  Every scale shares the SAME 7
significant taps; the scale only sets a per-channel delay wl_c in
{64,194,...,2014,2048} (17 distinct values).  The dense (64ch x 2048-tap)
convolution therefore collapses to:

    out[c, n] = sum_{k=0}^{6} w_c[k] * sigp[n + 2048 - wl_c + k]

with sigp = [zeros(2048), signal].  Folding (delay, tap) pairs into one
contraction axis of 17*7 = 119 <= 128 rows makes each 512-wide output block
a single K=119 matmul: lhsT[7d+k, c] = w_c[k] (if delay(c)==d else 0),
rhs[7d+k, n] = sigp[n0 + n + 2048 - delay_d + k].

Sharding: sequence-parallel over L.  Core r handles n in [512r, 512(r+1))
for all 4 batches and all 64 (re,im) channels.

Perf notes (from perfetto): the 16 chip DMA engines are shared by all 8
cores and per-transfer fixed costs (issue ~0.6us, ring startup ~0.8us,
sem wakeup ~0.5us) dominate over bytes, so everything is bf16 (gate is
2e-2, bf16 gives ~4e-3) and uses few fat-descriptor DMAs: 2 inputs
(b0+bank | b1b2b3), 2 outputs gated on the two PSUM->SBUF cast copies.
Matmuls pack even batches into PSUM partitions 0-63 and odd into 64-127
(tile_position) so each cast moves [128, 512].  The DRAM out tensor keeps
the packed [128, 1024] layout; the host unpacks.  Engine block-exit
drains cover output-DMA completion (no s_out semaphore round-trip).
"""

import os
import numpy as np
import ml_dtypes

import concourse.bacc as bacc
import concourse.bass as bass
from concourse import mybir
from concourse.bass_utils import run_bass_kernel_spmd

# ---------------------------------------------------------------- constants
B = 4
L = 4096
N_SCALES = 32
WLMAX = 2048
NCORES = 8
NBLK = L // NCORES          # 512 output columns per core
T = 7                       # taps kept per wavelet (tap 7 is 2.3e-11)
NCH = 2 * N_SCALES          # 64: [re x32, im x32]

_WLS = [64, 194, 324, 454, 584, 714, 844, 974, 1104, 1234, 1364, 1494,
        1624, 1754, 1884, 2014] + [2048] * 16
DELAYS = _WLS[:16] + [2048]          # 17 distinct
NDELAY = len(DELAYS)                 # 17
K_ROWS = NDELAY * T                  # 119 contraction rows (no dead rows)

# matmul dtype: "bfloat16" (1 cyc/row, rel err ~4e-3) or "float32" (exact)
MM_DTYPE = os.environ.get("CWT_MM_DTYPE", "bfloat16")
_NP_DT = {"bfloat16": ml_dtypes.bfloat16, "float32": np.float32,
          "float32r": np.float32}


def _wavelet_taps():
    t = np.arange(T, dtype=np.float32)
    env = np.exp(-0.5 * t * t).astype(np.float32)
    ph = np.float32(2.0 * np.pi * 1.0 / 6.0) * t
    wr = (env * np.cos(ph)).astype(np.float32)
    wi = (env * np.sin(ph)).astype(np.float32)
    return wr, wi


def _build_lhsT():
    """[119, 64] stationary operand: row 7d+k, col c -> w_c[k]."""
    wr, wi = _wavelet_taps()
    lhsT = np.zeros((K_ROWS, NCH), np.float32)
    for sc in range(N_SCALES):
        d = sc if sc < 16 else 16
        for k in range(T):
            lhsT[T * d + k, sc] = wr[k]
            lhsT[T * d + k, N_SCALES + sc] = wi[k]
    return lhsT


def _build_rhs_per_core(signal):
    """Per-core [119, B*512] moving operands (im2col over (delay, tap))."""
    sigp = np.zeros((B, WLMAX + L), np.float32)
    sigp[:, WLMAX:] = signal
    rhs_all = []
    for r in range(NCORES):
        rhs = np.zeros((K_ROWS, B * NBLK), np.float32)
        for d in range(NDELAY):
            s0 = WLMAX + NBLK * r - DELAYS[d]
            for b in range(B):
                for k in range(T):
                    rhs[T * d + k, NBLK * b:NBLK * (b + 1)] = \
                        sigp[b, s0 + k: s0 + k + NBLK]
        rhs_all.append(rhs)
    return rhs_all


def _build_nc():
    dt_mm = getattr(mybir.dt, MM_DTYPE)
    dt_out = mybir.dt.bfloat16 if MM_DTYPE == "bfloat16" else mybir.dt.float32
    nc = bacc.Bacc("TRN2", target_bir_lowering=False, debug=False,
                   num_devices=NCORES)
    # rhs layout: [b0 (512) | lhsT (64) | b1 | b2 | b3]
    rhs_d = nc.dram_tensor("rhs", [K_ROWS, B * NBLK + NCH], dt_mm,
                           kind="ExternalInput")
    # packed output: partition 64*(b%2)+c, col 512*(b//2)+n; host unpacks
    out_d = nc.dram_tensor("out", [2 * NCH, 2 * NBLK], dt_out,
                           kind="ExternalOutput")

    c1 = NBLK + NCH                       # end of chunk A (b0 + lhsT)
    offs = [0, c1, c1 + NBLK, c1 + 2 * NBLK]   # rhs col base per batch
    with (
        nc.sbuf_tensor("rhs_sb", [K_ROWS, B * NBLK + NCH], dt_mm) as rhs_sb,
        nc.sbuf_tensor("out_sb", [2 * NCH, 2 * NBLK], dt_out) as out_sb,
        nc.psum_tensor("acc", [2 * NCH, 2, NBLK], mybir.dt.float32) as acc,
        nc.semaphore("s_ina") as s_ina,
        nc.semaphore("s_inb") as s_inb,
        nc.semaphore("s_mm") as s_mm,
        nc.semaphore("s_cp") as s_cp,
        nc.semaphore("s_out") as s_out,
        nc.Block() as block,
    ):
        # Input DMAs issued BEFORE the Block (bare engine calls) so they
        # enter the streams right after the bass preamble.  Two fat
        # transfers: A = b0 + wavelet bank (gates mm0), B = b1b2b3.
        nc.sync.dma_start(
            rhs_sb[:, 0:c1], rhs_d[:, 0:c1]).then_inc(s_ina, 16)
        nc.scalar.dma_start(
            rhs_sb[:, c1:offs[3] + NBLK],
            rhs_d[:, c1:offs[3] + NBLK]).then_inc(s_inb, 16)

        @block.sync
        def _(sync):
            sync.wait_ge(s_cp, 1)
            sync.dma_start(
                out_d[:, 0:NBLK], out_sb[:, 0:NBLK]).then_inc(s_out, 16)

        @block.scalar
        def _(scalar):
            scalar.wait_ge(s_cp, 2)
            scalar.dma_start(out_d[:, NBLK:2 * NBLK],
                             out_sb[:, NBLK:2 * NBLK]).then_inc(s_out, 16)

        @block.tensor
        def _(tensor):
            lhsT_ap = rhs_sb[:, NBLK:NBLK + NCH]
            gates = [(s_ina, 16), (s_inb, 16), (s_inb, 16), (s_inb, 16)]
            for b in range(B):
                tensor.wait_ge(*gates[b])
                nc.tensor.matmul(
                    acc[NCH * (b % 2):NCH * (b % 2) + NCH, b // 2, :],
                    lhsT_ap,
                    rhs_sb[:, offs[b]:offs[b] + NBLK],
                    start=True, stop=True,
                ).then_inc(s_mm, 1)

        @block.vector
        def _(vector):
            for h in range(2):
                vector.wait_ge(s_mm, 2 * (h + 1))
                vector.tensor_copy(
                    out_sb[:, bass.ts(h, NBLK)], acc[:, h, :]
                ).then_inc(s_cp, 1)

    nc.compile()
    return nc


_NC_CACHE = {}


def _get_nc():
    key = MM_DTYPE
    if key not in _NC_CACHE:
        _NC_CACHE[key] = _build_nc()
    return _NC_CACHE[key]


def run(signal, trace=False, **spmd_kwargs):
    """Returns (out complex64 (4,32,4096), BassKernelResults)."""
    signal = np.asarray(signal, dtype=np.float32)
    assert signal.shape == (B, L)
    nc = _get_nc()
    np_dt = _NP_DT[MM_DTYPE]
    lhsT = _build_lhsT()
    rhs_all = _build_rhs_per_core(signal)
    packed = [np.concatenate(
        [r[:, :NBLK], lhsT, r[:, NBLK:]], axis=1).astype(np_dt)
        for r in rhs_all]
    in_maps = [{"rhs": packed[r]} for r in range(NCORES)]
    res = run_bass_kernel_spmd(nc, in_maps, core_ids=list(range(NCORES)),
                               trace=trace, **spmd_kwargs)
    out = np.empty((B, N_SCALES, L), np.complex64)
    for r in range(NCORES):
        o = np.asarray(res.results[r]["out"], np.float32)  # [128, 1024]
        sl = slice(NBLK * r, NBLK * (r + 1))
        for b in range(B):
            blk = o[NCH * (b % 2):NCH * (b % 2) + NCH,
                    NBLK * (b // 2):NBLK * (b // 2) + NBLK]
            out[b, :, sl] = blk[:N_SCALES] + 1j * blk[N_SCALES:]
    return out, res


def kernel(signal):
    out, _ = run(signal, trace=False)
    return out


# revision 8
# speedup vs baseline: 1.0772x; 1.0772x over previous
"""ContinuousWaveletTransform (Morlet bank, 32 scales) on 8 TRN2 cores.

Key structure exploited: the reference wavelet is w[k] = exp(-0.5 k^2) *
exp(i 2pi k / 6) (bandwidth=1), so the envelope underflows to exactly 0.0f
after k=14, and taps k>=7 are < 2.3e-11.  Every scale shares the SAME 7
significant taps; the scale only sets a per-channel delay wl_c in
{64,194,...,2014,2048} (17 distinct values).  The dense (64ch x 2048-tap)
convolution therefore collapses to:

    out[c, n] = sum_{k=0}^{6} w_c[k] * sigp[n + 2048 - wl_c + k]

with sigp = [zeros(2048), signal].  Folding (delay, tap) pairs into one
contraction axis of 17*7 = 119 <= 128 rows makes each 512-wide output block
a single K=119 matmul: lhsT[7d+k, c] = w_c[k] (if delay(c)==d else 0),
rhs[7d+k, n] = sigp[n0 + n + 2048 - delay_d + k].

Sharding: sequence-parallel over L.  Core r handles n in [512r, 512(r+1))
for all 4 batches and all 64 (re,im) channels.

Perf notes (from perfetto): the 16 chip DMA engines are shared by all 8
cores and per-transfer fixed costs (issue ~0.6us, ring startup ~0.8us,
sem wakeup ~0.5us) dominate over bytes, so everything is bf16 (gate is
2e-2, bf16 gives ~4e-3) and uses few fat-descriptor DMAs: 2 inputs
(b0+bank | b1b2b3), 2 outputs gated on the two PSUM->SBUF cast copies.
Matmuls pack even batches into PSUM partitions 0-63 and odd into 64-127
(tile_position) so each cast moves [128, 512].  The DRAM out tensor keeps
the packed [128, 1024] layout; the host unpacks.  Engine block-exit
drains cover output-DMA completion (no s_out semaphore round-trip).
"""

import os
import numpy as np
import ml_dtypes

import concourse.bacc as bacc
import concourse.bass as bass
from concourse import mybir
from concourse.bass_utils import run_bass_kernel_spmd

# ---------------------------------------------------------------- constants
B = 4
L = 4096
N_SCALES = 32
WLMAX = 2048
NCORES = 8
NBLK = L // NCORES          # 512 output columns per core
# taps kept per wavelet: env[k]=exp(-k^2/2) = 1, .61, .14, .011, 3.4e-4...
# T=4 truncation error ~2.5e-4 of output scale, far below bf16 noise.
T = int(os.environ.get("CWT_TAPS", "4"))
NCH = 2 * N_SCALES          # 64: [re x32, im x32]

_WLS = [64, 194, 324, 454, 584, 714, 844, 974, 1104, 1234, 1364, 1494,
        1624, 1754, 1884, 2014] + [2048] * 16
DELAYS = _WLS[:16] + [2048]          # 17 distinct
NDELAY = len(DELAYS)                 # 17
K_ROWS = NDELAY * T                  # 119 contraction rows (no dead rows)

# matmul dtype: "bfloat16" (1 cyc/row, rel err ~4e-3) or "float32" (exact)
MM_DTYPE = os.environ.get("CWT_MM_DTYPE", "bfloat16")
_NP_DT = {"bfloat16": ml_dtypes.bfloat16, "float32": np.float32,
          "float32r": np.float32}


def _wavelet_taps():
    t = np.arange(T, dtype=np.float32)
    env = np.exp(-0.5 * t * t).astype(np.float32)
    ph = np.float32(2.0 * np.pi * 1.0 / 6.0) * t
    wr = (env * np.cos(ph)).astype(np.float32)
    wi = (env * np.sin(ph)).astype(np.float32)
    return wr, wi


def _build_lhsT():
    """[119, 64] stationary operand: row 7d+k, col c -> w_c[k]."""
    wr, wi = _wavelet_taps()
    lhsT = np.zeros((K_ROWS, NCH), np.float32)
    for sc in range(N_SCALES):
        d = sc if sc < 16 else 16
        for k in range(T):
            lhsT[T * d + k, sc] = wr[k]
            lhsT[T * d + k, N_SCALES + sc] = wi[k]
    return lhsT


def _build_rhs_per_core(signal):
    """Per-core [119, B*512] moving operands (im2col over (delay, tap))."""
    sigp = np.zeros((B, WLMAX + L), np.float32)
    sigp[:, WLMAX:] = signal
    rhs_all = []
    for r in range(NCORES):
        rhs = np.zeros((K_ROWS, B * NBLK), np.float32)
        for d in range(NDELAY):
            s0 = WLMAX + NBLK * r - DELAYS[d]
            for b in range(B):
                for k in range(T):
                    rhs[T * d + k, NBLK * b:NBLK * (b + 1)] = \
                        sigp[b, s0 + k: s0 + k + NBLK]
        rhs_all.append(rhs)
    return rhs_all


def _build_nc():
    dt_mm = getattr(mybir.dt, MM_DTYPE)
    dt_out = mybir.dt.bfloat16 if MM_DTYPE == "bfloat16" else mybir.dt.float32
    nc = bacc.Bacc("TRN2", target_bir_lowering=False, debug=False,
                   num_devices=NCORES)
    # rhs layout: [b0 (512) | lhsT (64) | b1 | b2 | b3]
    rhs_d = nc.dram_tensor("rhs", [K_ROWS, B * NBLK + NCH], dt_mm,
                           kind="ExternalInput")
    # packed output: partition 64*(b%2)+c, col 512*(b//2)+n; host unpacks
    out_d = nc.dram_tensor("out", [2 * NCH, 2 * NBLK], dt_out,
                           kind="ExternalOutput")

    c1 = NBLK + NCH                       # end of chunk A (b0 + lhsT)
    offs = [0, c1, c1 + NBLK, c1 + 2 * NBLK]   # rhs col base per batch
    with (
        nc.sbuf_tensor("rhs_sb", [K_ROWS, B * NBLK + NCH], dt_mm) as rhs_sb,
        nc.sbuf_tensor("out_sb", [2 * NCH, 2 * NBLK], dt_out) as out_sb,
        nc.psum_tensor("acc", [2 * NCH, 2, NBLK], mybir.dt.float32) as acc,
        nc.semaphore("s_in0") as s_in0,
        nc.semaphore("s_in1") as s_in1,
        nc.semaphore("s_in2") as s_in2,
        nc.semaphore("s_in3") as s_in3,
        nc.semaphore("s_mm") as s_mm,
        nc.semaphore("s_cp") as s_cp,
        nc.semaphore("s_out") as s_out,
        nc.Block() as block,
    ):
        s_in = [s_in0, s_in1, s_in2, s_in3]
        # Input DMAs issued BEFORE the Block (bare engine calls) so they
        # enter the streams right after the bass preamble.  Per-batch
        # chunks (~1KB descriptors are the DMA-engine sweet spot); batch b
        # gates matmul b.
        nc.sync.dma_start(
            rhs_sb[:, 0:c1], rhs_d[:, 0:c1]).then_inc(s_in0, 16)
        nc.scalar.dma_start(
            rhs_sb[:, offs[1]:offs[1] + NBLK],
            rhs_d[:, offs[1]:offs[1] + NBLK]).then_inc(s_in1, 16)
        nc.sync.dma_start(
            rhs_sb[:, offs[2]:offs[2] + NBLK],
            rhs_d[:, offs[2]:offs[2] + NBLK]).then_inc(s_in2, 16)
        nc.scalar.dma_start(
            rhs_sb[:, offs[3]:offs[3] + NBLK],
            rhs_d[:, offs[3]:offs[3] + NBLK]).then_inc(s_in3, 16)

        @block.sync
        def _(sync):
            sync.wait_ge(s_cp, 1)
            sync.dma_start(
                out_d[:, 0:NBLK], out_sb[:, 0:NBLK]).then_inc(s_out, 16)

        @block.scalar
        def _(scalar):
            scalar.wait_ge(s_cp, 2)
            scalar.dma_start(out_d[:, NBLK:2 * NBLK],
                             out_sb[:, NBLK:2 * NBLK]).then_inc(s_out, 16)

        @block.tensor
        def _(tensor):
            lhsT_ap = rhs_sb[:, NBLK:NBLK + NCH]
            for b in range(B):
                tensor.wait_ge(s_in[b], 16)
                nc.tensor.matmul(
                    acc[NCH * (b % 2):NCH * (b % 2) + NCH, b // 2, :],
                    lhsT_ap,
                    rhs_sb[:, offs[b]:offs[b] + NBLK],
                    start=True, stop=True,
                ).then_inc(s_mm, 1)

        @block.vector
        def _(vector):
            for h in range(2):
                vector.wait_ge(s_mm, 2 * (h + 1))
                vector.tensor_copy(
                    out_sb[:, bass.ts(h, NBLK)], acc[:, h, :]
                ).then_inc(s_cp, 1)

    nc.compile()
    return nc


_NC_CACHE = {}


def _get_nc():
    key = MM_DTYPE
    if key not in _NC_CACHE:
        _NC_CACHE[key] = _build_nc()
    return _NC_CACHE[key]


def run(signal, trace=False, **spmd_kwargs):
    """Returns (out complex64 (4,32,4096), BassKernelResults)."""
    signal = np.asarray(signal, dtype=np.float32)
    assert signal.shape == (B, L)
    nc = _get_nc()
    np_dt = _NP_DT[MM_DTYPE]
    lhsT = _build_lhsT()
    rhs_all = _build_rhs_per_core(signal)
    packed = [np.concatenate(
        [r[:, :NBLK], lhsT, r[:, NBLK:]], axis=1).astype(np_dt)
        for r in rhs_all]
    in_maps = [{"rhs": packed[r]} for r in range(NCORES)]
    res = run_bass_kernel_spmd(nc, in_maps, core_ids=list(range(NCORES)),
                               trace=trace, **spmd_kwargs)
    out = np.empty((B, N_SCALES, L), np.complex64)
    for r in range(NCORES):
        o = np.asarray(res.results[r]["out"], np.float32)  # [128, 1024]
        sl = slice(NBLK * r, NBLK * (r + 1))
        for b in range(B):
            blk = o[NCH * (b % 2):NCH * (b % 2) + NCH,
                    NBLK * (b // 2):NBLK * (b // 2) + NBLK]
            out[b, :, sl] = blk[:N_SCALES] + 1j * blk[N_SCALES:]
    return out, res


def kernel(signal):
    out, _ = run(signal, trace=False)
    return out


# revision 11
# speedup vs baseline: 1.1113x; 1.0316x over previous
"""ContinuousWaveletTransform (Morlet bank, 32 scales) on 8 TRN2 cores.

Key structure exploited: the reference wavelet is w[k] = exp(-0.5 k^2) *
exp(i 2pi k / 6) (bandwidth=1), so the envelope underflows to exactly 0.0f
after k=14, and taps k>=7 are < 2.3e-11.  Every scale shares the SAME 7
significant taps; the scale only sets a per-channel delay wl_c in
{64,194,...,2014,2048} (17 distinct values).  The dense (64ch x 2048-tap)
convolution therefore collapses to:

    out[c, n] = sum_{k=0}^{6} w_c[k] * sigp[n + 2048 - wl_c + k]

with sigp = [zeros(2048), signal].  Folding (delay, tap) pairs into one
contraction axis of 17*7 = 119 <= 128 rows makes each 512-wide output block
a single K=119 matmul: lhsT[7d+k, c] = w_c[k] (if delay(c)==d else 0),
rhs[7d+k, n] = sigp[n0 + n + 2048 - delay_d + k].

Sharding: sequence-parallel over L.  Core r handles n in [512r, 512(r+1))
for all 4 batches and all 64 (re,im) channels.

Perf notes (from perfetto): the 16 chip DMA engines are shared by all 8
cores and per-transfer fixed costs (issue ~0.6us, ring startup ~0.8us,
sem wakeup ~0.5us) dominate over bytes, so everything is bf16 (gate is
2e-2, bf16 gives ~4e-3) and uses few fat-descriptor DMAs: 2 inputs
(b0+bank | b1b2b3), 2 outputs gated on the two PSUM->SBUF cast copies.
Matmuls pack even batches into PSUM partitions 0-63 and odd into 64-127
(tile_position) so each cast moves [128, 512].  The DRAM out tensor keeps
the packed [128, 1024] layout; the host unpacks.  Engine block-exit
drains cover output-DMA completion (no s_out semaphore round-trip).
"""

import os
import numpy as np
import ml_dtypes

import concourse.bacc as bacc
import concourse.bass as bass
from concourse import mybir
from concourse.bass_utils import run_bass_kernel_spmd

# ---------------------------------------------------------------- constants
B = 4
L = 4096
N_SCALES = 32
WLMAX = 2048
NCORES = 8
NBLK = L // NCORES          # 512 output columns per core
# taps kept per wavelet: env[k]=exp(-k^2/2) = 1, .61, .14, .011, 3.4e-4...
# T=4 truncation error ~2.5e-4 of output scale, far below bf16 noise.
T = int(os.environ.get("CWT_TAPS", "4"))
NCH = 2 * N_SCALES          # 64: [re x32, im x32]

_WLS = [64, 194, 324, 454, 584, 714, 844, 974, 1104, 1234, 1364, 1494,
        1624, 1754, 1884, 2014] + [2048] * 16
DELAYS = _WLS[:16] + [2048]          # 17 distinct
NDELAY = len(DELAYS)                 # 17
K_ROWS = NDELAY * T                  # 119 contraction rows (no dead rows)

# matmul dtype: "bfloat16" (1 cyc/row, rel err ~4e-3) or "float32" (exact)
MM_DTYPE = os.environ.get("CWT_MM_DTYPE", "bfloat16")
_NP_DT = {"bfloat16": ml_dtypes.bfloat16, "float32": np.float32,
          "float32r": np.float32}


def _wavelet_taps():
    t = np.arange(T, dtype=np.float32)
    env = np.exp(-0.5 * t * t).astype(np.float32)
    ph = np.float32(2.0 * np.pi * 1.0 / 6.0) * t
    wr = (env * np.cos(ph)).astype(np.float32)
    wi = (env * np.sin(ph)).astype(np.float32)
    return wr, wi


def _build_lhsT():
    """[119, 64] stationary operand: row 7d+k, col c -> w_c[k]."""
    wr, wi = _wavelet_taps()
    lhsT = np.zeros((K_ROWS, NCH), np.float32)
    for sc in range(N_SCALES):
        d = sc if sc < 16 else 16
        for k in range(T):
            lhsT[T * d + k, sc] = wr[k]
            lhsT[T * d + k, N_SCALES + sc] = wi[k]
    return lhsT


def _build_rhs_per_core(signal):
    """Per-core [119, B*512] moving operands (im2col over (delay, tap))."""
    sigp = np.zeros((B, WLMAX + L), np.float32)
    sigp[:, WLMAX:] = signal
    rhs_all = []
    for r in range(NCORES):
        rhs = np.zeros((K_ROWS, B * NBLK), np.float32)
        for d in range(NDELAY):
            s0 = WLMAX + NBLK * r - DELAYS[d]
            for b in range(B):
                for k in range(T):
                    rhs[T * d + k, NBLK * b:NBLK * (b + 1)] = \
                        sigp[b, s0 + k: s0 + k + NBLK]
        rhs_all.append(rhs)
    return rhs_all


def _build_nc():
    dt_mm = getattr(mybir.dt, MM_DTYPE)
    dt_out = mybir.dt.bfloat16 if MM_DTYPE == "bfloat16" else mybir.dt.float32
    nc = bacc.Bacc("TRN2", target_bir_lowering=False, debug=False,
                   num_devices=NCORES)
    # rhs layout: [b0 (512) | lhsT (64) | b1 | b2 | b3]
    rhs_d = nc.dram_tensor("rhs", [K_ROWS, B * NBLK + NCH], dt_mm,
                           kind="ExternalInput")
    # packed output: partition 64*(b%2)+c, col 512*(b//2)+n; host unpacks
    out_d = nc.dram_tensor("out", [2 * NCH, 2 * NBLK], dt_out,
                           kind="ExternalOutput")

    c1 = NBLK + NCH                       # end of chunk A (b0 + lhsT)
    offs = [0, c1, c1 + NBLK, c1 + 2 * NBLK]   # rhs col base per batch
    with (
        nc.sbuf_tensor("rhs_sb", [K_ROWS, B * NBLK + NCH], dt_mm) as rhs_sb,
        nc.sbuf_tensor("out_sb", [2 * NCH, 2 * NBLK], dt_out) as out_sb,
        nc.psum_tensor("acc", [2 * NCH, 2, NBLK], mybir.dt.float32) as acc,
        nc.semaphore("s_in0") as s_in0,
        nc.semaphore("s_in1") as s_in1,
        nc.semaphore("s_in2") as s_in2,
        nc.semaphore("s_in3") as s_in3,
        nc.semaphore("s_mm") as s_mm,
        nc.semaphore("s_cp") as s_cp,
        nc.semaphore("s_out") as s_out,
        nc.Block() as block,
    ):
        s_in = [s_in0, s_in1, s_in2, s_in3]
        # Input DMAs issued BEFORE the Block (bare engine calls) so they
        # enter the streams right after the bass preamble.  Per-batch
        # chunks (~1KB descriptors are the DMA-engine sweet spot); batch b
        # gates matmul b.
        nc.sync.dma_start(
            rhs_sb[:, 0:c1], rhs_d[:, 0:c1]).then_inc(s_in0, 16)
        nc.scalar.dma_start(
            rhs_sb[:, offs[1]:offs[1] + NBLK],
            rhs_d[:, offs[1]:offs[1] + NBLK]).then_inc(s_in1, 16)
        nc.sync.dma_start(
            rhs_sb[:, offs[2]:offs[2] + NBLK],
            rhs_d[:, offs[2]:offs[2] + NBLK]).then_inc(s_in2, 16)
        nc.scalar.dma_start(
            rhs_sb[:, offs[3]:offs[3] + NBLK],
            rhs_d[:, offs[3]:offs[3] + NBLK]).then_inc(s_in3, 16)

        @block.sync
        def _(sync):
            sync.wait_ge(s_cp, 1)
            sync.dma_start(
                out_d[:, 0:NBLK], out_sb[:, 0:NBLK]).then_inc(s_out, 16)

        @block.scalar
        def _(scalar):
            scalar.wait_ge(s_cp, 2)
            scalar.dma_start(out_d[:, NBLK:2 * NBLK],
                             out_sb[:, NBLK:2 * NBLK]).then_inc(s_out, 16)

        @block.tensor
        def _(tensor):
            lhsT_ap = rhs_sb[:, NBLK:NBLK + NCH]
            for b in range(B):
                tensor.wait_ge(s_in[b], 16)
                nc.tensor.matmul(
                    acc[NCH * (b % 2):NCH * (b % 2) + NCH, b // 2, :],
                    lhsT_ap,
                    rhs_sb[:, offs[b]:offs[b] + NBLK],
                    start=True, stop=True,
                ).then_inc(s_mm, 1)

        @block.vector
        def _(vector):
            for h in range(2):
                vector.wait_ge(s_mm, 2 * (h + 1))
                vector.tensor_copy(
                    out_sb[:, bass.ts(h, NBLK)], acc[:, h, :]
                ).then_inc(s_cp, 1)

    nc.compile()
    return nc


_NC_CACHE = {}


def _get_nc():
    key = MM_DTYPE
    if key not in _NC_CACHE:
        _NC_CACHE[key] = _build_nc()
    return _NC_CACHE[key]


def run(signal, trace=False, **spmd_kwargs):
    """Returns (out complex64 (4,32,4096), BassKernelResults)."""
    signal = np.asarray(signal, dtype=np.float32)
    assert signal.shape == (B, L)
    nc = _get_nc()
    np_dt = _NP_DT[MM_DTYPE]
    lhsT = _build_lhsT()
    rhs_all = _build_rhs_per_core(signal)
    packed = [np.concatenate(
        [r[:, :NBLK], lhsT, r[:, NBLK:]], axis=1).astype(np_dt)
        for r in rhs_all]
    in_maps = [{"rhs": packed[r]} for r in range(NCORES)]
    res = run_bass_kernel_spmd(nc, in_maps, core_ids=list(range(NCORES)),
                               trace=trace, **spmd_kwargs)
    out = np.empty((B, N_SCALES, L), np.complex64)
    for r in range(NCORES):
        o = np.asarray(res.results[r]["out"], np.float32)  # [128, 1024]
        sl = slice(NBLK * r, NBLK * (r + 1))
        for b in range(B):
            blk = o[NCH * (b % 2):NCH * (b % 2) + NCH,
                    NBLK * (b // 2):NBLK * (b // 2) + NBLK]
            out[b, :, sl] = blk[:N_SCALES] + 1j * blk[N_SCALES:]
    return out, res


def kernel(signal):
    out, _ = run(signal, trace=False)
    return out


# revision 12
# speedup vs baseline: 1.1127x; 1.0013x over previous
"""ContinuousWaveletTransform (Morlet bank, 32 scales) on 8 TRN2 cores.

Key structure exploited: the reference wavelet is w[k] = exp(-0.5 k^2) *
exp(i 2pi k / 6) (bandwidth=1), so the envelope underflows to exactly 0.0f
after k=14, and taps k>=7 are < 2.3e-11.  Every scale shares the SAME 7
significant taps; the scale only sets a per-channel delay wl_c in
{64,194,...,2014,2048} (17 distinct values).  The dense (64ch x 2048-tap)
convolution therefore collapses to:

    out[c, n] = sum_{k=0}^{6} w_c[k] * sigp[n + 2048 - wl_c + k]

with sigp = [zeros(2048), signal].  Folding (delay, tap) pairs into one
contraction axis of 17*7 = 119 <= 128 rows makes each 512-wide output block
a single K=119 matmul: lhsT[7d+k, c] = w_c[k] (if delay(c)==d else 0),
rhs[7d+k, n] = sigp[n0 + n + 2048 - delay_d + k].

Sharding: sequence-parallel over L.  Core r handles n in [512r, 512(r+1))
for all 4 batches and all 64 (re,im) channels.

Perf notes (from perfetto): the 16 chip DMA engines are shared by all 8
cores and per-transfer fixed costs (issue ~0.6us, ring startup ~0.8us,
sem wakeup ~0.5us) dominate over bytes, so everything is bf16 (gate is
2e-2, bf16 gives ~4e-3) and uses few fat-descriptor DMAs: 2 inputs
(b0+bank | b1b2b3), 2 outputs gated on the two PSUM->SBUF cast copies.
Matmuls pack even batches into PSUM partitions 0-63 and odd into 64-127
(tile_position) so each cast moves [128, 512].  The DRAM out tensor keeps
the packed [128, 1024] layout; the host unpacks.  Engine block-exit
drains cover output-DMA completion (no s_out semaphore round-trip).
"""

import os
import numpy as np
import ml_dtypes

import concourse.bacc as bacc
import concourse.bass as bass
from concourse import mybir
from concourse.bass_utils import run_bass_kernel_spmd

# ---------------------------------------------------------------- constants
B = 4
L = 4096
N_SCALES = 32
WLMAX = 2048
NCORES = 8
NBLK = L // NCORES          # 512 output columns per core
# taps kept per wavelet: env[k]=exp(-k^2/2) = 1, .61, .14, .011, 3.4e-4...
# T=4 truncation error ~2.5e-4 of output scale, far below bf16 noise.
T = int(os.environ.get("CWT_TAPS", "4"))
NCH = 2 * N_SCALES          # 64: [re x32, im x32]

_WLS = [64, 194, 324, 454, 584, 714, 844, 974, 1104, 1234, 1364, 1494,
        1624, 1754, 1884, 2014] + [2048] * 16
DELAYS = _WLS[:16] + [2048]          # 17 distinct
NDELAY = len(DELAYS)                 # 17
K_ROWS = NDELAY * T                  # 119 contraction rows (no dead rows)

# matmul dtype: "bfloat16" (1 cyc/row, rel err ~4e-3) or "float32" (exact)
MM_DTYPE = os.environ.get("CWT_MM_DTYPE", "bfloat16")
_NP_DT = {"bfloat16": ml_dtypes.bfloat16, "float32": np.float32,
          "float32r": np.float32}


def _wavelet_taps():
    t = np.arange(T, dtype=np.float32)
    env = np.exp(-0.5 * t * t).astype(np.float32)
    ph = np.float32(2.0 * np.pi * 1.0 / 6.0) * t
    wr = (env * np.cos(ph)).astype(np.float32)
    wi = (env * np.sin(ph)).astype(np.float32)
    return wr, wi


def _build_lhsT():
    """[119, 64] stationary operand: row 7d+k, col c -> w_c[k]."""
    wr, wi = _wavelet_taps()
    lhsT = np.zeros((K_ROWS, NCH), np.float32)
    for sc in range(N_SCALES):
        d = sc if sc < 16 else 16
        for k in range(T):
            lhsT[T * d + k, sc] = wr[k]
            lhsT[T * d + k, N_SCALES + sc] = wi[k]
    return lhsT


def _build_rhs_per_core(signal):
    """Per-core [119, B*512] moving operands (im2col over (delay, tap))."""
    sigp = np.zeros((B, WLMAX + L), np.float32)
    sigp[:, WLMAX:] = signal
    rhs_all = []
    for r in range(NCORES):
        rhs = np.zeros((K_ROWS, B * NBLK), np.float32)
        for d in range(NDELAY):
            s0 = WLMAX + NBLK * r - DELAYS[d]
            for b in range(B):
                for k in range(T):
                    rhs[T * d + k, NBLK * b:NBLK * (b + 1)] = \
                        sigp[b, s0 + k: s0 + k + NBLK]
        rhs_all.append(rhs)
    return rhs_all


def _build_nc():
    dt_mm = getattr(mybir.dt, MM_DTYPE)
    dt_out = mybir.dt.bfloat16 if MM_DTYPE == "bfloat16" else mybir.dt.float32
    nc = bacc.Bacc("TRN2", target_bir_lowering=False, debug=False,
                   num_devices=NCORES)
    # rhs layout: [b0 (512) | lhsT (64) | b1 | b2 | b3]
    rhs_d = nc.dram_tensor("rhs", [K_ROWS, B * NBLK + NCH], dt_mm,
                           kind="ExternalInput")
    # packed output: partition 64*(b%2)+c, col 512*(b//2)+n; host unpacks
    out_d = nc.dram_tensor("out", [2 * NCH, 2 * NBLK], dt_out,
                           kind="ExternalOutput")

    c1 = NBLK + NCH                       # end of chunk A (b0 + lhsT)
    offs = [0, c1, c1 + NBLK, c1 + 2 * NBLK]   # rhs col base per batch
    with (
        nc.sbuf_tensor("rhs_sb", [K_ROWS, B * NBLK + NCH], dt_mm) as rhs_sb,
        nc.sbuf_tensor("out_sb", [2 * NCH, 2 * NBLK], dt_out) as out_sb,
        nc.psum_tensor("acc", [2 * NCH, 2, NBLK], mybir.dt.float32) as acc,
        nc.semaphore("s_in0") as s_in0,
        nc.semaphore("s_in1") as s_in1,
        nc.semaphore("s_in2") as s_in2,
        nc.semaphore("s_in3") as s_in3,
        nc.semaphore("s_mm") as s_mm,
        nc.semaphore("s_cp") as s_cp,
        nc.semaphore("s_out") as s_out,
    ):
        s_in = [s_in0, s_in1, s_in2, s_in3]
        # Everything is emitted bare (no Block): no block entry/exit
        # barriers or branches; walrus's own per-engine epilogue drains
        # cover output-DMA completion.  Per-batch input chunks (~1KB
        # descriptors are the DMA-engine sweet spot); batch b gates
        # matmul b.
        nc.sync.dma_start(
            rhs_sb[:, 0:c1], rhs_d[:, 0:c1]).then_inc(s_in0, 16)
        nc.scalar.dma_start(
            rhs_sb[:, offs[1]:offs[1] + NBLK],
            rhs_d[:, offs[1]:offs[1] + NBLK]).then_inc(s_in1, 16)
        nc.sync.dma_start(
            rhs_sb[:, offs[2]:offs[2] + NBLK],
            rhs_d[:, offs[2]:offs[2] + NBLK]).then_inc(s_in2, 16)
        nc.scalar.dma_start(
            rhs_sb[:, offs[3]:offs[3] + NBLK],
            rhs_d[:, offs[3]:offs[3] + NBLK]).then_inc(s_in3, 16)

        lhsT_ap = rhs_sb[:, NBLK:NBLK + NCH]
        for b in range(B):
            nc.tensor.wait_ge(s_in[b], 16)
            nc.tensor.matmul(
                acc[NCH * (b % 2):NCH * (b % 2) + NCH, b // 2, :],
                lhsT_ap,
                rhs_sb[:, offs[b]:offs[b] + NBLK],
                start=True, stop=True,
            ).then_inc(s_mm, 1)

        for h in range(2):
            nc.vector.wait_ge(s_mm, 2 * (h + 1))
            nc.vector.tensor_copy(
                out_sb[:, bass.ts(h, NBLK)], acc[:, h, :]
            ).then_inc(s_cp, 1)

        nc.sync.wait_ge(s_cp, 1)
        nc.sync.dma_start(
            out_d[:, 0:NBLK], out_sb[:, 0:NBLK]).then_inc(s_out, 16)
        nc.scalar.wait_ge(s_cp, 2)
        nc.scalar.dma_start(out_d[:, NBLK:2 * NBLK],
                            out_sb[:, NBLK:2 * NBLK]).then_inc(s_out, 16)

    nc.compile()
    return nc


_NC_CACHE = {}


def _get_nc():
    key = MM_DTYPE
    if key not in _NC_CACHE:
        _NC_CACHE[key] = _build_nc()
    return _NC_CACHE[key]


def run(signal, trace=False, **spmd_kwargs):
    """Returns (out complex64 (4,32,4096), BassKernelResults)."""
    signal = np.asarray(signal, dtype=np.float32)
    assert signal.shape == (B, L)
    nc = _get_nc()
    np_dt = _NP_DT[MM_DTYPE]
    lhsT = _build_lhsT()
    rhs_all = _build_rhs_per_core(signal)
    packed = [np.concatenate(
        [r[:, :NBLK], lhsT, r[:, NBLK:]], axis=1).astype(np_dt)
        for r in rhs_all]
    in_maps = [{"rhs": packed[r]} for r in range(NCORES)]
    res = run_bass_kernel_spmd(nc, in_maps, core_ids=list(range(NCORES)),
                               trace=trace, **spmd_kwargs)
    out = np.empty((B, N_SCALES, L), np.complex64)
    for r in range(NCORES):
        o = np.asarray(res.results[r]["out"], np.float32)  # [128, 1024]
        sl = slice(NBLK * r, NBLK * (r + 1))
        for b in range(B):
            blk = o[NCH * (b % 2):NCH * (b % 2) + NCH,
                    NBLK * (b // 2):NBLK * (b // 2) + NBLK]
            out[b, :, sl] = blk[:N_SCALES] + 1j * blk[N_SCALES:]
    return out, res


def kernel(signal):
    out, _ = run(signal, trace=False)
    return out
